# revision 4
# baseline (speedup 1.0000x reference)
"""Trainium2 Bass kernel for nn_AbstractFullyConnected (DeepPoly-style abstract MLP).

Network: 784 -> 4096 -> 4096 -> 4096 -> 10, batch=1, with box-bound propagation.

Math reformulation (exact, no approximation):
  For any W and bounds (low, high), with c=(low+high)/2, r=(high-low)/2:
      W_pos@low + W_neg@high = W@c - |W|@r
      W_pos@high + W_neg@low = W@c + |W|@r
  So each layer needs two matvec passes: W @ [x, c] and |W| @ r.
  After each AbstractRelu boundary low=0, so c' = r' = high'/2.

Sharding: layers 1-3 row-sharded (output dim) across 8 cores, 512 rows each.
Each core holds W.T slices pre-transposed on the host, streams them through
the PE as the *moving* operand with the tiny vectors [x, c, c] / [0, -r, r]
as the 3-column *stationary* operand, accumulating (z, u, v) = (W@x, W@c,
|W|@r) in PSUM rows 0-2.  |W| is computed on-chip (DVE/ACT) from W so HBM
traffic stays at one pass over the weights.  Bias is folded in as an extra
contraction row with stationary (1, 1, 0).  After layers 1 and 2 the
(z, u, v) shards are AllGathered and every core redundantly computes the
elementwise ReLU-boundary on the full 4096-vector.  Layer 3's raw (z, u, v)
shards are the kernel output; the final boundary and the tiny 10x4096
layer 4 run on the host in numpy.
"""

import numpy as np

N_CORES = 8
MEAN = np.float32(0.1307)
STD = np.float32(0.3081)
EPS = np.float32(1e-6)
MS = 512          # output rows per core, layers 1-3
K1, K1P, T1 = 784, 896, 7
K23, T = 4096, 32
G = 8             # DMA chunks per big weight matrix
CH = T * MS // G  # 2048 columns per chunk (4 tau groups)

_CACHE = {}


def _build_nc():
    import concourse.bacc as bacc
    import concourse.mybir as mybir
    import concourse.tile as tile

    F32 = mybir.dt.float32
    ALU = mybir.AluOpType
    ACTF = mybir.ActivationFunctionType

    nc = bacc.Bacc("TRN2", target_bir_lowering=False, debug=False,
                   num_devices=N_CORES)

    stat1_d = nc.dram_tensor("stat1", [128, 42], F32, kind="ExternalInput")
    brow_d = nc.dram_tensor("brow", [1, 1027], F32, kind="ExternalInput")
    wt1_d = nc.dram_tensor("wt1", [128, T1 * MS], F32, kind="ExternalInput")
    wt2_d = nc.dram_tensor("wt2", [128, T * MS], F32, kind="ExternalInput")
    wt3_d = nc.dram_tensor("wt3", [128, T * MS], F32, kind="ExternalInput")
    out_d = nc.dram_tensor("out", [3, MS], F32, kind="ExternalOutput")

    with tile.TileContext(nc) as tc:
        with (
            tc.tile_pool(name="wp", bufs=1) as wp,
            tc.tile_pool(name="sp", bufs=1) as sp,
            tc.tile_pool(name="absp", bufs=8) as absp,
            tc.tile_pool(name="pp", bufs=1, space="PSUM") as pp,
            tc.tile_pool(name="dp", bufs=1, space="DRAM") as dp,
        ):
            # ---- input DMAs (HWDGE / SP ring; issue order == drain order) ----
            stat1 = sp.tile([128, 42], F32, tag="stat1")
            brow = sp.tile([1, 1027], F32, tag="brow")
            nc.sync.dma_start(stat1[:], stat1_d[:])
            nc.sync.dma_start(brow[:], brow_d[:])
            wt1 = wp.tile([128, T1 * MS], F32, tag="wt1")
            nc.sync.dma_start(wt1[:], wt1_d[:])
            wt2 = [wp.tile([128, CH], F32, name=f"wt2_{g}", tag=f"wt2_{g}") for g in range(G)]
            wt3 = [wp.tile([128, CH], F32, name=f"wt3_{g}", tag=f"wt3_{g}") for g in range(G)]
            for g in range(G):
                nc.sync.dma_start(wt2[g][:], wt2_d[:, g * CH:(g + 1) * CH])
            for g in range(G):
                nc.sync.dma_start(wt3[g][:], wt3_d[:, g * CH:(g + 1) * CH])

            def abs_chunk(src_ap, idx):
                # |W| tile computed on-chip; alternate DVE/ACT for throughput
                t_ = absp.tile([128, MS], F32, name=f"abs_{idx}", tag="abs")
                if idx % 2 == 0:
                    # |x| = max(x * -1, x) on DVE
                    nc.vector.scalar_tensor_tensor(t_[:], src_ap, -1.0, src_ap,
                                                   ALU.mult, ALU.max)
                else:
                    nc.scalar.activation(t_[:], src_ap, ACTF.Abs)
                return t_

            def layer(wstat_fn, rstat_fn, wtiles, tpg, psum_tag, bias_off):
                """One abstract layer: psum rows = (W@x, W@c + b, |W|@r)."""
                ps = pp.tile([3, MS], F32, tag=psum_tag)
                ntau = len(wtiles) * tpg
                for t_ in range(ntau):
                    g, j = divmod(t_, tpg)
                    nc.tensor.matmul(ps[:], wstat_fn(t_),
                                     wtiles[g][:, j * MS:(j + 1) * MS],
                                     start=(t_ == 0), stop=False)
                if bias_off is not None:
                    nc.tensor.matmul(ps[:], brow[0:1, 1024:1027],
                                     brow[0:1, bias_off:bias_off + MS],
                                     start=False, stop=False)
                for t_ in range(ntau):
                    g, j = divmod(t_, tpg)
                    a = abs_chunk(wtiles[g][:, j * MS:(j + 1) * MS], t_)
                    nc.tensor.matmul(ps[:], rstat_fn(t_), a[:],
                                     start=False, stop=(t_ == ntau - 1))
                return ps

            def boundary(ps, idx):
                """PSUM (z,u,v) shard -> AllGather -> full-vector ReLU boundary
                -> next layer's stationaries (wstat, rstat)."""
                sb_out = sp.tile([3, MS], F32, tag=f"sbout{idx}")
                nc.scalar.activation(sb_out[:], ps[:], ACTF.Copy)
                b_in = dp.tile([3, MS], F32, tag=f"bin{idx}")
                ag = dp.tile([N_CORES, 3, 16, T], F32, tag=f"ag{idx}")
                nc.gpsimd.dma_start(b_in[:], sb_out[:])
                nc.gpsimd.collective_compute(
                    "AllGather", mybir.AluOpType.bypass,
                    replica_groups=[list(range(N_CORES))],
                    ins=[b_in[:]], outs=[ag[:]],
                )
                # gather to [128, 96]: partition p=(core,p16), cols (vec, tau)
                vec = sp.tile([128, 3 * T], F32, tag=f"vec{idx}")
                for v in range(3):
                    nc.gpsimd.dma_start(vec[:, v * T:(v + 1) * T], ag[:, v, :, :])
                X = vec[:, 0:T]          # z + b   (concrete x)
                L = vec[:, T:2 * T]      # low  = W@c + b - |W|@r
                H = vec[:, 2 * T:3 * T]  # high = W@c + b + |W|@r

                wstat = sp.tile([128, 3 * T], F32, tag=f"wstat{idx}")
                rstat = sp.tile([128, 3 * T], F32, tag=f"rstat{idx}")
                tmp = {n: sp.tile([128, T], F32, name=f"{n}{idx}", tag=f"{n}{idx}")
                       for n in ("d0", "d1", "r0", "r1", "u1", "u2", "s")}
                nc.vector.memset(rstat[:], 0.0)
                nc.vector.tensor_sub(tmp["d0"][:], H, L)
                nc.vector.tensor_scalar_add(tmp["d1"][:], tmp["d0"][:], float(EPS))
                nc.vector.reciprocal(tmp["r1"][:], tmp["d1"][:])
                nc.vector.reciprocal(tmp["r0"][:], tmp["d0"][:])
                nc.vector.tensor_mul(tmp["u1"][:], H, tmp["r1"][:])
                nc.vector.tensor_mul(tmp["u2"][:], L, tmp["r0"][:])
                nc.vector.tensor_add(tmp["s"][:], tmp["u1"][:], tmp["u2"][:])
                s = tmp["s"][:]
                # high' = H * (H*r1 + L*r0);  c' = r' = high'/2
                nc.vector.tensor_relu(wstat[:, 0:3 * T:3], X)
                stt = nc.vector.scalar_tensor_tensor
                stt(wstat[:, 1:3 * T:3], H, 0.5, s, ALU.mult, ALU.mult)
                stt(wstat[:, 2:3 * T:3], H, 0.5, s, ALU.mult, ALU.mult)
                stt(rstat[:, 2:3 * T:3], H, 0.5, s, ALU.mult, ALU.mult)
                stt(rstat[:, 1:3 * T:3], H, -0.5, s, ALU.mult, ALU.mult)
                return wstat, rstat

            ps1 = layer(lambda t_: stat1[:, 3 * t_:3 * t_ + 3],
                        lambda t_: stat1[:, 21 + 3 * t_:21 + 3 * t_ + 3],
                        [wt1], T1, "ps1", 0)
            wstat2, rstat2 = boundary(ps1, 1)
            ps2 = layer(lambda t_: wstat2[:, 3 * t_:3 * t_ + 3],
                        lambda t_: rstat2[:, 3 * t_:3 * t_ + 3],
                        wt2, T // G, "ps2", 512)
            wstat3, rstat3 = boundary(ps2, 2)
            ps3 = layer(lambda t_: wstat3[:, 3 * t_:3 * t_ + 3],
                        lambda t_: rstat3[:, 3 * t_:3 * t_ + 3],
                        wt3, T // G, "ps3", None)
            sb3 = sp.tile([3, MS], F32, tag="sb3")
            nc.scalar.activation(sb3[:], ps3[:], ACTF.Copy)
            nc.gpsimd.dma_start(out_d[:], sb3[:])
    nc.compile()
    return nc


def _w_prep(shard, kp):
    """(512, K) row-shard of W -> device layout [128, T*512] with
    wt[p, tau*512 + m] = W.T[p*T + tau, m], zero-padded to kp rows."""
    k = shard.shape[1]
    arr = np.zeros((kp, MS), np.float32)
    arr[:k] = np.ascontiguousarray(shard.T)
    t = kp // 128
    return np.ascontiguousarray(arr.reshape(128, t, MS).reshape(128, t * MS))


def _prep_inputs(x, low, high, Ws, bs):
    xn = (x - MEAN) / STD
    ln = (low - MEAN) / STD
    hn = (high - MEAN) / STD
    c0 = (ln + hn) * np.float32(0.5)
    r0 = (hn - ln) * np.float32(0.5)

    def padv(v):
        p = np.zeros(K1P, np.float32)
        p[:K1] = v
        return p.reshape(128, T1)

    xs, cs, rs = padv(xn), padv(c0), padv(r0)
    stat1 = np.zeros((128, 42), np.float32)
    stat1[:, 0:21:3] = xs
    stat1[:, 1:21:3] = cs
    stat1[:, 2:21:3] = cs
    stat1[:, 22:42:3] = -rs
    stat1[:, 23:42:3] = rs

    in_maps = []
    for c in range(N_CORES):
        sl = slice(c * MS, (c + 1) * MS)
        brow = np.zeros((1, 1027), np.float32)
        brow[0, 0:512] = bs[0][sl]
        brow[0, 512:1024] = bs[1][sl]
        brow[0, 1024:1027] = (1.0, 1.0, 1.0)
        in_maps.append({
            "stat1": stat1,
            "brow": brow,
            "wt1": _w_prep(Ws[0][sl], K1P),
            "wt2": _w_prep(Ws[1][sl], K23),
            "wt3": _w_prep(Ws[2][sl], K23),
        })
    return in_maps


def _run_device(in_maps, trace=False):
    from concourse.bass_utils import run_bass_kernel_spmd

    if "nc" not in _CACHE:
        _CACHE["nc"] = _build_nc()
    return run_bass_kernel_spmd(_CACHE["nc"], in_maps,
                                core_ids=list(range(N_CORES)), trace=trace)


def kernel(x, low, high, Ws, bs, _trace=False, _res_out=None):
    x = np.asarray(x, np.float32).reshape(-1)
    low = np.asarray(low, np.float32).reshape(-1)
    high = np.asarray(high, np.float32).reshape(-1)
    Ws = [np.ascontiguousarray(np.asarray(W, np.float32)) for W in Ws]
    bs = [np.asarray(b, np.float32) for b in bs]

    in_maps = _prep_inputs(x, low, high, Ws, bs)
    res = _run_device(in_maps, trace=_trace)
    if _res_out is not None:
        _res_out.append(res)

    outs = [res.results[c]["out"] for c in range(N_CORES)]
    z3 = np.concatenate([o[0] for o in outs])
    lo3 = np.concatenate([o[1] for o in outs])
    hi3 = np.concatenate([o[2] for o in outs])

    # layer-3 bias + ReLU boundary (host, fp32)
    b3 = bs[2]
    x3 = z3 + b3
    lo = lo3 + b3
    hi = hi3 + b3
    d0 = hi - lo
    slope = hi / (d0 + EPS)
    ubint = lo * hi / d0
    hi2 = slope * hi + ubint
    x3p = np.maximum(x3, np.float32(0))
    c3 = hi2 * np.float32(0.5)

    # layer 4 (10 x 4096) on host
    W4, b4 = Ws[3], bs[3]
    z4 = W4 @ x3p
    u4 = W4 @ c3
    v4 = np.abs(W4) @ c3
    x_out = (z4 + b4).reshape(1, -1).astype(np.float32)
    low_out = (u4 - v4 + b4).astype(np.float32)
    high_out = (u4 + v4 + b4).astype(np.float32)
    return (x_out, low_out, high_out)
